# revision 16
# baseline (speedup 1.0000x reference)
"""Trainium2 Bass kernel for nn_CausalSelfAttention2 (landmark/grouped causal attention).

Sharding: data-parallel over batch B=8 across 8 NeuronCores (1 batch element per core).
Each core computes qkv = x_b @ W_attn, grouped causal attention with mean-pooled
landmark tokens, landmark-level attention, and the output projection, entirely
on-chip (weights resident in SBUF, one pass over x).

Self-contained: hardcodes all shapes; no sibling imports.
"""

import os

import numpy as np

import concourse.bass as bass
import concourse.bacc as bacc
import concourse.mybir as mybir
from concourse import tile
from concourse.bass_utils import run_bass_kernel_spmd

FP = mybir.dt.float32
FPR = mybir.dt.float32r
AX = mybir.AxisListType
ALU = mybir.AluOpType
ACTF = mybir.ActivationFunctionType

B, T, C = 8, 2048, 1024
H, D = 16, 64
G, GT = 8, 256
L = GT + 1   # 257 tokens per group incl. landmark
LP = L + 1   # padded to even for fp32r matmul ISA restrictions
N_CORES = 8
SCALE = 1.0 / 8.0  # 1/sqrt(D)


def _build_body(nc, tc, ctx):
    x_d = nc.dram_tensor("x", [T, C], FP, kind="ExternalInput")
    wa_d = nc.dram_tensor("wa", [C, 3 * C], FPR, kind="ExternalInput")
    wp_d = nc.dram_tensor("wp", [C, C], FPR, kind="ExternalInput")
    masks_d = nc.dram_tensor("masks", [2, 128, LP], FP, kind="ExternalInput")
    ident_d = nc.dram_tensor("ident", [128, 128], FP, kind="ExternalInput")
    identt_d = nc.dram_tensor("identt", [128, 64], FPR, kind="ExternalInput")
    lconst_d = nc.dram_tensor("lconst", [8, 16], FP, kind="ExternalInput")
    ones_d = nc.dram_tensor("ones", [128, 64], FPR, kind="ExternalInput")
    out_d = nc.dram_tensor("out", [T, C], FP, kind="ExternalOutput")
    yq_d = nc.dram_tensor("yq", [H, G - 1, D], FP, kind="ExternalOutput")
    yk_d = nc.dram_tensor("yk", [H, G - 1, D], FP, kind="ExternalOutput")
    yv_d = nc.dram_tensor("yv", [H, G - 1, D], FP, kind="ExternalOutput")

    x_r = x_d.rearrange("(g ti p) c -> g ti p c", g=G, p=128)
    out_r = out_d.rearrange("(g mi p) c -> g mi p c", g=G, p=128)

    # ---- persistent pools ----
    wpool = ctx.enter_context(tc.tile_pool(name="w", bufs=1))
    per = ctx.enter_context(tc.tile_pool(name="per", bufs=1))

    wa_sb = wpool.tile([128, 8, 3 * C], FPR)  # W_attn, c-tiled
    wp_sb = wpool.tile([128, 8, C], FPR)      # W_proj, c-tiled
    ident_sb = wpool.tile([128, 128], FP)
    identt_sb = wpool.tile([128, 64], FPR)
    masks_sb = wpool.tile([128, 2, LP], FP)
    lconst_sb = wpool.tile([8, 16], FP)      # [:, 0:8]=tril mask, [:, 8:16]=eye(8)
    ones_sb = wpool.tile([128, 64], FPR)

    nc.sync.dma_start(out=wa_sb[:], in_=wa_d.rearrange("(ci p) n -> p ci n", p=128))
    nc.sync.dma_start(out=wp_sb[:], in_=wp_d.rearrange("(ci p) n -> p ci n", p=128))
    nc.sync.dma_start(out=ident_sb[:], in_=ident_d[:])
    nc.sync.dma_start(out=identt_sb[:], in_=identt_d[:])
    nc.sync.dma_start(out=masks_sb[:], in_=masks_d.rearrange("s p l -> p s l"))
    nc.sync.dma_start(out=lconst_sb[:], in_=lconst_d[:])
    nc.sync.dma_start(out=ones_sb[:], in_=ones_d[:])

    # persistent small buffers for the landmark epilogue
    xmeanT = per.tile([128, 8, G], FPR)        # group means of x^T: [c, ci, g]
    qlT_buf = per.tile([128, 8, G], FPR)       # landmark q: [2 heads x 64, mi, g]
    klT_buf = per.tile([128, 8, G], FPR)
    nul_T = per.tile([64, H, G], FP)          # raw nu col 256 per (h, g)
    denl_buf = per.tile([8, H * G], FP)       # raw den col 256, replicated rows; col h*8+g
    vl_all = per.tile([8, C], FP)             # landmark v rows: [g, c]

    # ---- cycling pools ----
    xload = ctx.enter_context(tc.tile_pool(name="xload", bufs=1))
    xtp = ctx.enter_context(tc.tile_pool(name="xtp", bufs=1))
    qkp = ctx.enter_context(tc.tile_pool(name="qkp", bufs=2))
    vp = ctx.enter_context(tc.tile_pool(name="vp", bufs=1))
    ep = ctx.enter_context(tc.tile_pool(name="ep", bufs=2))
    xap = ctx.enter_context(tc.tile_pool(name="xap", bufs=1))
    osp = ctx.enter_context(tc.tile_pool(name="osp", bufs=1))
    rdp = ctx.enter_context(tc.tile_pool(name="rdp", bufs=2))
    smp = ctx.enter_context(tc.tile_pool(name="smp", bufs=1))

    psA = ctx.enter_context(tc.tile_pool(name="psA", bufs=2, space="PSUM"))
    psB = ctx.enter_context(tc.tile_pool(name="psB", bufs=2, space="PSUM"))
    psC = ctx.enter_context(tc.tile_pool(name="psC", bufs=2, space="PSUM"))
    psN = ctx.enter_context(tc.tile_pool(name="psN", bufs=1, space="PSUM"))
    psD = ctx.enter_context(tc.tile_pool(name="psD", bufs=1, space="PSUM"))

    n_groups = int(os.environ.get("KGROUPS", str(G)))
    phases = os.environ.get("KPHASES", "ABCDEF")
    for g in range(n_groups):
        # ---- phase A: load x rows for this group, transpose to x^T, compute mean ----
        xg = xload.tile([128, 2, C], FP, tag="xload")
        for ti in range(2):
            nc.sync.dma_start(out=xg[:, ti, :], in_=x_r[g, ti])
        xTg = xtp.tile([128, 8, LP], FPR, tag="xtp")
        for ci in range(8):
            for ti in range(2):
                ps = psC.tile([128, 128], FP, tag="C")
                nc.tensor.transpose(
                    ps[:], xg[:, ti, ci * 128:(ci + 1) * 128], ident_sb[:]
                )
                nc.vector.tensor_copy(xTg[:, ci, ti * 128:(ti + 1) * 128], ps[:])
            # group mean -> landmark column 256 (and persist for epilogue)
            with nc.allow_low_precision(reason="fp32r mean (fp32-width accumulate)"):
                nc.vector.tensor_reduce(
                    xTg[:, ci, GT:GT + 1], xTg[:, ci, 0:GT], axis=AX.X, op=ALU.add
                )
            nc.vector.tensor_scalar_mul(
                xTg[:, ci, GT:GT + 1], xTg[:, ci, GT:GT + 1], 1.0 / GT
            )
            nc.vector.tensor_copy(xTg[:, ci, GT + 1:GT + 2], xTg[:, ci, GT:GT + 1])
            nc.vector.tensor_copy(xmeanT[:, ci, g:g + 1], xTg[:, ci, GT:GT + 1])

        # ---- phase B: v = x_aug @ Wv in [t, c] layout ----
        if "B" not in phases:
            continue
        v_sb = vp.tile([128, 2, C], FPR, tag="vp")
        for ti in range(2):
            for nj in range(2):
                ps = psA.tile([128, 512], FP, tag="A")
                for ci in range(8):
                    nc.tensor.matmul(
                        ps[:],
                        xTg[:, ci, ti * 128:(ti + 1) * 128],
                        wa_sb[:, ci, 2 * C + nj * 512:2 * C + (nj + 1) * 512],
                        start=(ci == 0),
                        stop=(ci == 7),
                    )
                nc.vector.tensor_copy(v_sb[:, ti, nj * 512:(nj + 1) * 512], ps[:])

        # ---- phase C: per head-pair: q^T,k^T tiles then attention ----
        if "C" not in phases:
            continue
        xattT = xap.tile([128, 8, GT], FPR, tag="xap")
        for mi in range(8):
            qk = qkp.tile([128, 2, LP], FPR, tag="qkp")
            for j in range(2):  # 0 = q, 1 = k
                ps = psA.tile([128, LP], FP, tag="A")
                for ci in range(8):
                    nc.tensor.matmul(
                        ps[:],
                        wa_sb[:, ci, j * C + mi * 128:j * C + (mi + 1) * 128],
                        xTg[:, ci, :],
                        start=(ci == 0),
                        stop=(ci == 7),
                    )
                nc.vector.tensor_copy(qk[:, j, :], ps[:])
            # save landmark q/k columns for the epilogue
            nc.vector.tensor_copy(qlT_buf[:, mi, g:g + 1], qk[:, 0, GT:GT + 1])
            nc.vector.tensor_copy(klT_buf[:, mi, g:g + 1], qk[:, 1, GT:GT + 1])

            for hh in range(2):
                h = 2 * mi + hh
                po = hh * 64
                qa = qk[po:po + 64, 0, :]
                ka = qk[po:po + 64, 1, :]

                ps0 = psB.tile([128, LP], FP, tag="B")
                nc.tensor.matmul(
                    ps0[:], ka[:, 0:128], qa,
                    start=True, stop=True,
                )
                ps1 = psB.tile([128, LP], FP, tag="B")
                nc.tensor.matmul(
                    ps1[:], ka[:, 128:256], qa,
                    start=True, stop=True,
                )

                expm = ep.tile([128, 2, LP], FPR, tag="ep")
                nc.scalar.activation(expm[:, 0, :], ps0[:], ACTF.Exp, scale=SCALE)
                nc.scalar.activation(expm[:, 1, :], ps1[:], ACTF.Exp, scale=SCALE)
                nc.vector.tensor_mul(expm[:, 0, :], expm[:, 0, :], masks_sb[:, 0, :])
                nc.vector.tensor_mul(expm[:, 1, :], expm[:, 1, :], masks_sb[:, 1, :])

                ps_nu = psN.tile([64, LP], FP, tag="N")
                ps_den = psD.tile([64, LP], FP, tag="D")
                for si in range(2):
                    nc.tensor.matmul(
                        ps_nu[:],
                        v_sb[:, si, h * 64:(h + 1) * 64],
                        expm[:, si, :],
                        start=(si == 0), stop=(si == 1),
                    )
                for si in range(2):
                    nc.tensor.matmul(
                        ps_den[:],
                        ones_sb[:],
                        expm[:, si, :],
                        start=(si == 0), stop=(si == 1),
                    )

                rd = rdp.tile([64, L], FP, tag="rd")
                nc.vector.reciprocal(rd[:], ps_den[:, 0:L])
                nc.vector.tensor_mul(
                    xattT[po:po + 64, mi, :], ps_nu[:, 0:GT], rd[:, 0:GT]
                )
                # save raw landmark-column numerator / denominator for the epilogue
                nc.vector.tensor_copy(nul_T[:, h, g:g + 1], ps_nu[:, GT:GT + 1])
                nc.vector.tensor_copy(
                    denl_buf[:, h * G + g:h * G + g + 1], ps_den[0:8, GT:GT + 1]
                )

        # ---- phase D: output projection for this group's 256 tokens ----
        if "D" not in phases:
            continue
        for mi2 in range(2):
            out_sb = osp.tile([128, C], FP, tag="osp")
            for nj in range(2):
                ps = psC.tile([128, 512], FP, tag="C")
                for ci in range(8):
                    nc.tensor.matmul(
                        ps[:],
                        xattT[:, ci, mi2 * 128:(mi2 + 1) * 128],
                        wp_sb[:, ci, nj * 512:(nj + 1) * 512],
                        start=(ci == 0), stop=(ci == 7),
                    )
                nc.vector.tensor_copy(out_sb[:, nj * 512:(nj + 1) * 512], ps[:])
            nc.sync.dma_start(out=out_r[g, mi2], in_=out_sb[:])

    # ---- landmark v rows (all groups at once): vl = xmean @ Wv ----
    if "E" not in phases:
        return
    for nj in range(2):
        ps = psA.tile([8, 512], FP, tag="A")
        for ci in range(8):
            nc.tensor.matmul(
                ps[:],
                xmeanT[:, ci, :],
                wa_sb[:, ci, 2 * C + nj * 512:2 * C + (nj + 1) * 512],
                start=(ci == 0), stop=(ci == 7),
            )
        nc.vector.tensor_copy(vl_all[:, nj * 512:(nj + 1) * 512], ps[:])

    # ---- landmark epilogue: per-head attention over the first 7 group landmarks ----
    if "F" not in phases:
        return
    maskl = lconst_sb[:, 0:8]
    ident8 = lconst_sb[:, 8:16]
    for h in range(H):
        po = (h % 2) * 64
        mi = h // 2
        ql = qlT_buf[po:po + 64, mi, :]
        kl = klT_buf[po:po + 64, mi, :]

        # scores_l [t, s] = ql^T kl
        ps_l = psB.tile([8, 8], FP, tag="B")
        nc.tensor.matmul(ps_l[:], ql, kl, start=True, stop=True)
        expl = smp.tile([8, 8], FP, tag="expl")
        nc.scalar.activation(expl[:], ps_l[:], ACTF.Exp, scale=SCALE)
        nc.vector.tensor_mul(expl[:], expl[:], maskl)

        # el = diag(expl) (landmark self-attention weight), den_l = row sums
        scratch = smp.tile([8, 8], FP, tag="scr")
        el = smp.tile([8, 1], FP, tag="el")
        nc.vector.tensor_mul(scratch[:], expl[:], ident8)
        nc.vector.tensor_reduce(el[:], scratch[:], axis=AX.X, op=ALU.add)
        den_l = smp.tile([8, 1], FP, tag="denl")
        nc.vector.tensor_reduce(den_l[:], expl[:], axis=AX.X, op=ALU.add)

        # vl_in rows: complete landmark outputs of intra-group attention
        ps_nt = psC.tile([8, 64], FP, tag="C")
        nc.tensor.transpose(ps_nt[:], nul_T[:, h, :], ident_sb[0:64, 0:64])
        scr2 = smp.tile([8, 8], FP, tag="scr2")
        dl8 = smp.tile([8, 1], FP, tag="dl8")
        nc.vector.tensor_mul(scr2[:], denl_buf[:, h * G:h * G + G], ident8)
        nc.vector.tensor_reduce(dl8[:], scr2[:], axis=AX.X, op=ALU.add)
        dtot = smp.tile([8, 1], FP, tag="dtot")
        nc.vector.tensor_add(dtot[:], dl8[:], el[:])
        rtot = smp.tile([8, 1], FP, tag="rtot")
        nc.vector.reciprocal(rtot[:], dtot[:])
        vl_in = smp.tile([8, 64], FPR, tag="vlin")
        nc.vector.tensor_scalar(
            vl_in[:], vl_all[:, h * 64:(h + 1) * 64], el[:], None, op0=ALU.mult
        )
        nc.vector.tensor_add(vl_in[:], vl_in[:], ps_nt[:])
        nc.vector.tensor_scalar(vl_in[:], vl_in[:], rtot[:], None, op0=ALU.mult)

        # yv = softmax(scores_l) @ vl_in  (t-orientation; needs expl^T as lhsT)
        ps_et = psC.tile([8, 8], FP, tag="C")
        nc.tensor.transpose(ps_et[:], expl[:], ident_sb[0:8, 0:8])
        elT = smp.tile([8, 8], FPR, tag="elT")
        nc.vector.tensor_copy(elT[:], ps_et[:])
        ps_yv = psB.tile([8, 64], FP, tag="B")
        nc.tensor.matmul(
            ps_yv[:], elT[:], vl_in[:],
            start=True, stop=True,
        )
        rl = smp.tile([8, 1], FP, tag="rl")
        nc.vector.reciprocal(rl[0:7, :], den_l[0:7, :])
        yv_sb = smp.tile([7, 64], FP, tag="yvs")
        nc.vector.tensor_scalar(yv_sb[:], ps_yv[0:7, :], rl[0:7, :], None, op0=ALU.mult)
        nc.sync.dma_start(out=yv_d[h], in_=yv_sb[:])

        # yq / yk: transpose landmark q/k to [7, 64] and store
        ps_q = psC.tile([8, 64], FPR, tag="C")
        nc.tensor.transpose(ps_q[:], ql, identt_sb[po:po + 64, :])
        yq_sb = smp.tile([7, 64], FP, tag="yqs")
        nc.vector.tensor_copy(yq_sb[:], ps_q[0:7, :])
        nc.sync.dma_start(out=yq_d[h], in_=yq_sb[:])

        ps_k = psC.tile([8, 64], FPR, tag="C")
        nc.tensor.transpose(ps_k[:], kl, identt_sb[po:po + 64, :])
        yk_sb = smp.tile([7, 64], FP, tag="yks")
        nc.vector.tensor_copy(yk_sb[:], ps_k[0:7, :])
        nc.sync.dma_start(out=yk_d[h], in_=yk_sb[:])


_NC_CACHE = None


def build_nc():
    global _NC_CACHE
    if _NC_CACHE is not None:
        return _NC_CACHE
    from contextlib import ExitStack

    nc = bacc.Bacc("TRN2", target_bir_lowering=False, debug=False, num_devices=N_CORES)
    with tile.TileContext(nc) as tc:
        with ExitStack() as ctx:
            _build_body(nc, tc, ctx)
    nc.compile()
    _NC_CACHE = nc
    return nc


def host_constants():
    s_idx = np.arange(128)[:, None]
    t_idx = np.arange(LP)[None, :]
    masks = np.zeros((2, 128, LP), dtype=np.float32)
    masks[0] = (t_idx >= s_idx).astype(np.float32)
    masks[1] = (t_idx >= s_idx + 128).astype(np.float32)
    masks[:, :, GT] = 1.0   # landmark query attends everything
    masks[:, :, GT + 1] = 0.0  # pad column contributes nothing
    ident = np.eye(128, dtype=np.float32)
    identt = np.vstack([np.eye(64, dtype=np.float32)] * 2)
    lconst = np.zeros((8, 16), dtype=np.float32)
    lconst[:, 0:8] = np.tril(np.ones((8, 8), dtype=np.float32))
    lconst[:, 8:16] = np.eye(8, dtype=np.float32)
    ones = np.ones((128, 64), dtype=np.float32)
    return masks, ident, identt, lconst, ones


def kernel(x, W_attn, W_proj):
    x = np.ascontiguousarray(np.asarray(x, dtype=np.float32))
    W_attn = np.ascontiguousarray(np.asarray(W_attn, dtype=np.float32))
    W_proj = np.ascontiguousarray(np.asarray(W_proj, dtype=np.float32))
    nc = build_nc()
    masks, ident, identt, lconst, ones = host_constants()
    in_maps = [
        {
            "x": x[b],
            "wa": W_attn,
            "wp": W_proj,
            "masks": masks,
            "ident": ident,
            "identt": identt,
            "lconst": lconst,
            "ones": ones,
        }
        for b in range(B)
    ]
    res = run_bass_kernel_spmd(nc, in_maps, list(range(N_CORES)))
    out = np.stack([res.results[b]["out"] for b in range(B)])
    yq = np.stack([res.results[b]["yq"] for b in range(B)]).reshape(B, H, G - 1, 1, D)
    yk = np.stack([res.results[b]["yk"] for b in range(B)]).reshape(B, H, G - 1, 1, D)
    yv = np.stack([res.results[b]["yv"] for b in range(B)]).reshape(B, H, G - 1, 1, D)
    return out, yq, yk, yv
